# Initial kernel scaffold
#
"""Luong attention (method='general') scores for batch — TRN2 Bass kernel.

Reference computation (jax):
    proj   = einsum('sbh,oh->sbo', encoder_outputs, attn_w) + attn_b   # [S,B,H]
    scores = einsum('bh,sbh->bs', hidden[0], proj)                      # [B,S]
    attn   = softmax(scores, axis=1)                                    # [B,S]

Algebraic rewrite used here:
    scores[b,s] = sum_h enc[s,b,h] * q[b,h] + hidden[b]·attn_b
    with q = hidden[0] @ attn_w.
The bias term is constant in s, so it cancels in the softmax and is dropped.
This turns a 137-GFLOP matmul problem into a single streaming pass over
encoder_outputs (256 MB) with a fused multiply+reduce on the vector engine.

Sharding: data-parallel over batch. Core i handles batches [4i, 4i+4), gets
its enc shard [S, 4, H], the transposed hidden shard [H, 4], and a replicated
attn_w. Each core computes its own softmax (no collectives) and writes
attn [4, S].
"""

import numpy as np

import concourse.bacc as bacc
import concourse.bass_isa as bass_isa
import concourse.mybir as mybir
import concourse.tile as tile
from concourse.bass_utils import run_bass_kernel_spmd
from concourse.masks import make_identity

F32 = mybir.dt.float32

S, B, H = 2048, 32, 1024
NCORES = 8
BL = B // NCORES        # batches per core = 4
T = S // 128            # s-chunks of 128 = 16
OC = H // 128           # contraction chunks for q = 8

_CACHE: dict = {}


def _build_program():
    nc = bacc.Bacc(
        "TRN2",
        target_bir_lowering=False,
        debug=False,
        enable_asserts=True,
        num_devices=NCORES,
    )
    enc = nc.dram_tensor("enc", [S, BL, H], F32, kind="ExternalInput").ap()
    w = nc.dram_tensor("w", [H, H], F32, kind="ExternalInput").ap()
    hidt = nc.dram_tensor("hidt", [H, BL], F32, kind="ExternalInput").ap()
    out = nc.dram_tensor("out", [BL, S], F32, kind="ExternalOutput").ap()

    with tile.TileContext(nc) as tc:
        with (
            tc.tile_pool(name="consts", bufs=1) as consts,
            tc.tile_pool(name="wpool", bufs=1) as wpool,
            tc.tile_pool(name="encp", bufs=4) as encp,
            tc.tile_pool(name="prodp", bufs=2) as prodp,
            tc.tile_pool(name="small", bufs=1) as small,
            tc.tile_pool(name="psq", bufs=1, space="PSUM") as psq,
            tc.tile_pool(name="psr", bufs=2, space="PSUM") as psr,
            tc.tile_pool(name="pst", bufs=1, space="PSUM") as pst,
        ):
            # ---- load weights & hidden^T -------------------------------
            w_sb = wpool.tile([128, OC, H], F32)
            nc.sync.dma_start(out=w_sb, in_=w.rearrange("(c p) h -> p c h", p=128))
            ht_sb = wpool.tile([128, OC, BL], F32)
            nc.sync.dma_start(out=ht_sb, in_=hidt.rearrange("(c p) b -> p c b", p=128))

            # ---- q = hidden @ W  ([BL, H] in PSUM) ---------------------
            q_ps = psq.tile([BL, H], F32)
            for half in range(2):
                for c in range(OC):
                    nc.tensor.matmul(
                        q_ps[:, half * 512 : (half + 1) * 512],
                        ht_sb[:, c, :],
                        w_sb[:, c, half * 512 : (half + 1) * 512],
                        start=(c == 0),
                        stop=(c == OC - 1),
                    )
            q_sb = small.tile([BL, H], F32)
            nc.scalar.copy(out=q_sb, in_=q_ps)

            # ---- replicate q across all 128 partitions -----------------
            # sel[:, j*128:(j+1)*128] is a [BL, 128] selector with row j all
            # ones; matmul(sel_j, q_sb) broadcasts q row j to 128 partitions.
            sel = consts.tile([BL, BL * 128], F32)
            nc.vector.memset(sel, 0.0)
            for j in range(BL):
                nc.vector.memset(sel[j : j + 1, j * 128 : (j + 1) * 128], 1.0)
            qrep = consts.tile([128, BL, H], F32)
            for j in range(BL):
                r_ps = psr.tile([128, 1024], F32, tag="rps")
                for half in range(2):
                    nc.tensor.matmul(
                        r_ps[:, half * 512 : (half + 1) * 512],
                        sel[:, j * 128 : (j + 1) * 128],
                        q_sb[:, half * 512 : (half + 1) * 512],
                        start=True,
                        stop=True,
                    )
                nc.scalar.copy(out=qrep[:, j, :], in_=r_ps)

            identity = consts.tile([128, 128], F32)
            make_identity(nc, identity)

            # ---- main streaming pass: scores[s, (b,t)] -----------------
            scores = small.tile([128, BL * T], F32)
            for t in range(T):
                enc_t = encp.tile([128, BL, H], F32)
                nc.sync.dma_start(out=enc_t, in_=enc[t * 128 : (t + 1) * 128, :, :])
                for j in range(BL):
                    prod = prodp.tile([128, H], F32)
                    nc.vector.tensor_tensor_reduce(
                        out=prod,
                        in0=enc_t[:, j, :],
                        in1=qrep[:, j, :],
                        scale=1.0,
                        scalar=0.0,
                        op0=mybir.AluOpType.mult,
                        op1=mybir.AluOpType.add,
                        accum_out=scores[:, j * T + t : j * T + t + 1],
                    )

            # ---- softmax over s (per batch) ----------------------------
            # per-partition max over the T chunk-columns of each batch
            pmax = small.tile([128, BL], F32)
            nc.vector.tensor_reduce(
                out=pmax,
                in_=scores.rearrange("p (j t) -> p j t", t=T),
                axis=mybir.AxisListType.X,
                op=mybir.AluOpType.max,
            )
            bmax = small.tile([128, BL], F32)
            nc.gpsimd.partition_all_reduce(
                bmax, pmax, channels=128, reduce_op=bass_isa.ReduceOp.max
            )
            probs = small.tile([128, BL * T], F32)
            esum = small.tile([128, BL], F32)
            for j in range(BL):
                sl = slice(j * T, (j + 1) * T)
                nc.vector.tensor_scalar(
                    out=probs[:, sl],
                    in0=scores[:, sl],
                    scalar1=bmax[:, j : j + 1],
                    scalar2=None,
                    op0=mybir.AluOpType.subtract,
                )
                nc.scalar.activation(
                    out=probs[:, sl],
                    in_=probs[:, sl],
                    func=mybir.ActivationFunctionType.Exp,
                    accum_out=esum[:, j : j + 1],
                )
            dsum = small.tile([128, BL], F32)
            nc.gpsimd.partition_all_reduce(
                dsum, esum, channels=128, reduce_op=bass_isa.ReduceOp.add
            )
            rsum = small.tile([128, BL], F32)
            nc.vector.reciprocal(out=rsum, in_=dsum)
            attn = small.tile([128, BL * T], F32)
            for j in range(BL):
                sl = slice(j * T, (j + 1) * T)
                nc.vector.tensor_scalar_mul(
                    out=attn[:, sl], in0=probs[:, sl], scalar1=rsum[:, j : j + 1]
                )

            # ---- transpose [s_local, (b,t)] -> [(b,t), s_local], store -
            at_ps = pst.tile([BL * T, 128], F32)
            nc.tensor.transpose(at_ps, attn, identity)
            at_sb = small.tile([BL * T, 128], F32)
            nc.scalar.copy(out=at_sb, in_=at_ps)
            nc.sync.dma_start(
                out=out.rearrange("b (t s) -> (b t) s", s=128), in_=at_sb
            )

    nc.compile()
    return nc


def kernel(hidden, encoder_outputs, attn_w, attn_b):
    if "nc" not in _CACHE:
        _CACHE["nc"] = _build_program()
    nc = _CACHE["nc"]

    hidden = np.asarray(hidden, dtype=np.float32)
    encoder_outputs = np.asarray(encoder_outputs, dtype=np.float32)
    attn_w = np.asarray(attn_w, dtype=np.float32)

    # torch-Linear convention: proj = enc @ W^T, so q = hidden @ W (contraction
    # over W's rows). hidt[o, b] = hidden[0, b, o].
    in_maps = []
    for i in range(NCORES):
        bs = slice(i * BL, (i + 1) * BL)
        in_maps.append(
            {
                "enc": np.ascontiguousarray(encoder_outputs[:, bs, :]),
                "w": attn_w,
                "hidt": np.ascontiguousarray(hidden[0, bs, :].T),
            }
        )

    res = run_bass_kernel_spmd(nc, in_maps, core_ids=list(range(NCORES)))
    attn = np.concatenate([res.results[i]["out"] for i in range(NCORES)], axis=0)
    return attn[None].astype(np.float32)


# revision 9
# speedup vs baseline: 1.1623x; 1.1623x over previous
"""Luong attention (method='general') scores for batch — TRN2 Bass kernel.

Reference computation (jax):
    proj   = einsum('sbh,oh->sbo', encoder_outputs, attn_w) + attn_b   # [S,B,H]
    scores = einsum('bh,sbh->bs', hidden[0], proj)                      # [B,S]
    attn   = softmax(scores, axis=1)                                    # [B,S]

Algebraic rewrite used here:
    scores[b,s] = sum_h enc[s,b,h] * q[b,h] + hidden[b]·attn_b
    with q = hidden[0] @ attn_w  (computed on host: 67 MFLOP of prep vs the
    reference's 137 GFLOP, which this rewrite eliminates entirely).
The bias term is constant in s, so it cancels in the softmax and is dropped.
The device kernel is a single streaming pass over encoder_outputs (256 MB):
an elementwise multiply on the vector engine fused with per-batch reductions
on the scalar engine (activation Copy + accum_out), then an on-chip softmax.

Sharding: data-parallel over batch. Core i handles batches [4i, 4i+4): it
gets enc shard [S, 4, H] and q shard [4, H], computes its own softmax (no
collectives), and writes attn [4, S].
"""

import numpy as np

import concourse.bacc as bacc
import concourse.bass as bass
import concourse.bass_isa as bass_isa
import concourse.mybir as mybir
import concourse.tile as tile
from concourse.bass_utils import run_bass_kernel_spmd
from concourse.masks import make_identity

F32 = mybir.dt.float32

S, B, H = 2048, 32, 1024
NCORES = 8
BL = B // NCORES        # batches per core = 4
T = S // 128            # s-chunks of 128 = 16
TPT = 2                 # s-chunks per DMA tile
NT = T // TPT           # DMA tiles = 8

_CACHE: dict = {}


def _build_program():
    nc = bacc.Bacc(
        "TRN2",
        target_bir_lowering=False,
        debug=False,
        enable_asserts=True,
        num_devices=NCORES,
    )
    enc = nc.dram_tensor("enc", [S, BL, H], F32, kind="ExternalInput").ap()
    q = nc.dram_tensor("q", [BL, H], F32, kind="ExternalInput").ap()
    out = nc.dram_tensor("out", [BL, S], F32, kind="ExternalOutput").ap()

    with tile.TileContext(nc) as tc:
        with (
            tc.tile_pool(name="consts", bufs=1) as consts,
            tc.tile_pool(name="encp", bufs=3) as encp,
            tc.tile_pool(name="prodp", bufs=2) as prodp,
            tc.tile_pool(name="small", bufs=1) as small,
            tc.tile_pool(name="pst", bufs=1, space="PSUM") as pst,
        ):
            # ---- broadcast q to all 128 partitions (SWDGE replication) -
            qrep = consts.tile([128, BL, H], F32)
            q_bcast = bass.AP(
                tensor=q.tensor, offset=0, ap=[[0, 128], [H, BL], [1, H]]
            )
            nc.gpsimd.dma_start(out=qrep, in_=q_bcast)

            identity = consts.tile([128, 128], F32)
            make_identity(nc, identity)

            # ---- main streaming pass: scores[s, (b,t)] -----------------
            # DVE does the elementwise multiply; ScalarE reduces over h via
            # activation(Copy, accum_out) so the two engines pipeline.
            scores = small.tile([128, BL * T], F32)
            for it in range(NT):
                enc_t = encp.tile([128, TPT, BL, H], F32)
                nc.sync.dma_start(
                    out=enc_t,
                    in_=enc[it * 128 * TPT : (it + 1) * 128 * TPT, :, :].rearrange(
                        "(c p) b h -> p c b h", p=128
                    ),
                )
                prod = prodp.tile([128, TPT, BL, H], F32)
                for c in range(TPT):
                    nc.vector.tensor_mul(
                        out=prod[:, c], in0=enc_t[:, c], in1=qrep
                    )
                for c in range(TPT):
                    t = it * TPT + c
                    for j in range(BL):
                        nc.scalar.activation(
                            out=prod[:, c, j, :],
                            in_=prod[:, c, j, :],
                            func=mybir.ActivationFunctionType.Copy,
                            accum_out=scores[:, j * T + t : j * T + t + 1],
                        )

            # ---- softmax over s (per batch) ----------------------------
            pmax = small.tile([128, BL], F32)
            nc.vector.tensor_reduce(
                out=pmax,
                in_=scores.rearrange("p (j t) -> p j t", t=T),
                axis=mybir.AxisListType.X,
                op=mybir.AluOpType.max,
            )
            bmax = small.tile([128, BL], F32)
            nc.gpsimd.partition_all_reduce(
                bmax, pmax, channels=128, reduce_op=bass_isa.ReduceOp.max
            )
            probs = small.tile([128, BL * T], F32)
            esum = small.tile([128, BL], F32)
            for j in range(BL):
                sl = slice(j * T, (j + 1) * T)
                nc.vector.tensor_scalar(
                    out=probs[:, sl],
                    in0=scores[:, sl],
                    scalar1=bmax[:, j : j + 1],
                    scalar2=None,
                    op0=mybir.AluOpType.subtract,
                )
                nc.scalar.activation(
                    out=probs[:, sl],
                    in_=probs[:, sl],
                    func=mybir.ActivationFunctionType.Exp,
                    accum_out=esum[:, j : j + 1],
                )
            dsum = small.tile([128, BL], F32)
            nc.gpsimd.partition_all_reduce(
                dsum, esum, channels=128, reduce_op=bass_isa.ReduceOp.add
            )
            rsum = small.tile([128, BL], F32)
            nc.vector.reciprocal(out=rsum, in_=dsum)
            attn = small.tile([128, BL * T], F32)
            for j in range(BL):
                sl = slice(j * T, (j + 1) * T)
                nc.vector.tensor_scalar_mul(
                    out=attn[:, sl], in0=probs[:, sl], scalar1=rsum[:, j : j + 1]
                )

            # ---- transpose [s_local, (b,t)] -> [(b,t), s_local], store -
            at_ps = pst.tile([BL * T, 128], F32)
            nc.tensor.transpose(at_ps, attn, identity)
            at_sb = small.tile([BL * T, 128], F32)
            nc.scalar.copy(out=at_sb, in_=at_ps)
            nc.sync.dma_start(
                out=out.rearrange("b (t s) -> (b t) s", s=128), in_=at_sb
            )

    nc.compile()
    return nc


def _shard_inputs(hidden, encoder_outputs, attn_w):
    # torch-Linear convention: proj = enc @ W^T, so q = hidden @ W
    # (contraction over W's rows).
    qfull = (hidden[0].astype(np.float32) @ attn_w.astype(np.float32)).astype(
        np.float32
    )
    in_maps = []
    for i in range(NCORES):
        bs = slice(i * BL, (i + 1) * BL)
        in_maps.append(
            {
                "enc": np.ascontiguousarray(encoder_outputs[:, bs, :]),
                "q": np.ascontiguousarray(qfull[bs, :]),
            }
        )
    return in_maps


def kernel(hidden, encoder_outputs, attn_w, attn_b):
    if "nc" not in _CACHE:
        _CACHE["nc"] = _build_program()
    nc = _CACHE["nc"]

    hidden = np.asarray(hidden, dtype=np.float32)
    encoder_outputs = np.asarray(encoder_outputs, dtype=np.float32)
    attn_w = np.asarray(attn_w, dtype=np.float32)

    in_maps = _shard_inputs(hidden, encoder_outputs, attn_w)
    res = run_bass_kernel_spmd(nc, in_maps, core_ids=list(range(NCORES)))
    attn = np.concatenate([res.results[i]["out"] for i in range(NCORES)], axis=0)
    return attn[None].astype(np.float32)
